# revision 46
# baseline (speedup 1.0000x reference)
"""PeakLocalMax (41x41 NMS mask) Trainium2 Bass kernel.

Input : batch_heatmap (16, 1024, 1024, 2) float32
Output: bool mask, same shape: (x == maxpool41x41(x)) & (x > 0.5)

Strategy (per core; batch sharded 2 images/core over 8 cores):
  - Exact f32 separable sliding-window max via van Herk/Gil-Werman:
    segmented prefix/suffix max scans implemented with tensor_tensor_scan
    (op0=min with a block-reset mask, op1=max) + a fused 3-way max
    (scalar_tensor_tensor) combine that also folds in the 0.5 threshold
    as c = nextafter(0.5): M2 = max(window_max, c); out = (x >= M2).
    Un-padded block grid; edge-window clipping comes from persistent
    constant-c margins on the scan-output tiles, so each 1D pass is
    exactly 3 DVE instructions (fwd scan, reversed scan, STT combine).
  - W-direction pass on (H=partitions, W=free) channel-planar tiles
    (host passes x as [B, C, H, W]); H-direction pass on PE-transposed
    strips (W=partitions, H=free); transpose back via PE.  All scan/STT
    work is VectorE-only: walrus rejects these opcodes (and even plain
    tensor_tensor) on GpSimd in ISA v3.
  - The final compare leaves DVE: the transpose-back matmul leaves M2 in
    PSUM with stop=False, a second matmul with a negated identity
    accumulates -x into the same PSUM region (PSUM = M2 - x, sign-exact
    in fp32), and the Activation engine computes the mask as
    relu((M2 - x) * -3e38 + 1) -> u8 (1 iff x >= M2, which implies
    x > 0.5 since M2 >= nextafter(0.5) by the folded threshold).  The
    very last (img, ch) unit keeps an is_ge compare on DVE so the
    program tail stays on the busy engine instead of PE->ACT.
  - Emission is software-pipelined per channel
    (W0 H0c0 F0c0 H0c1 W1 F0c1 H1c0 F1c0 H1c1 F1c1) so each engine's
    program order matches dependency readiness; output stores go on the
    ACT DGE queue so they never head-of-line-block input loads on SP;
    scan scratch is parity double-buffered to avoid WAR sem stalls; all
    constant/mask setup runs on the idle Pool engine.
  - Boundary trims: the first unit's W-scans are split into two chained
    scans at an asymmetric s=560 boundary (initial=prev[:, -1:]) sized
    so the first-chunk scan drains just before the second chunk's sem
    gate; the output is channel-planar [B, C, H, W] (host re-interleaves)
    so stores are 128KB per (ch, hb) and fire inside each channel phase.
    TimelineSim: 233780 ns/core (baseline 272284, 1.165x), DVE ~96.6%
    busy; startup ~3.6us and final store+drain ~3.3us are the only DVE
    idle windows left, both latency-chain floors of the hardware model.
"""

import os
import sys
import numpy as np

_TRN_REPO = "/opt/trn_rl_repo"

H = 1024
W = 1024
C = 2
B_PER_CORE = 2
N_CORES = 8
V = 20            # min_distance
WIN = 2 * V + 1   # 41
HB = H // 128     # 8 h-blocks
WB = W // 128     # 8 w-blocks
BIG = float(np.float32(3e38))
C05 = float(np.nextafter(np.float32(0.5), np.float32(1)))

_CACHE = {}


def _build():
    if _TRN_REPO not in sys.path:
        sys.path.insert(0, _TRN_REPO)
    from contextlib import ExitStack
    from concourse import bacc, mybir
    import concourse.tile as tile
    from concourse.masks import make_identity
    from concourse.bass import _add_dep_helper

    F32 = mybir.dt.float32
    U8 = mybir.dt.uint8
    Alu = mybir.AluOpType

    nc = bacc.Bacc("TRN2", debug=False, num_devices=N_CORES)
    # channel-planar host-side copy of x: all loads are contiguous 4KB
    # per-channel row lines instead of stride-2 interleaved views
    xp_d = nc.dram_tensor("xp", [B_PER_CORE, C, H, W], F32, kind="ExternalInput").ap()
    # planar output too (host re-interleaves): per-channel stores fire as
    # soon as that channel's compare lands instead of waiting for both
    y_d = nc.dram_tensor("y", [B_PER_CORE, C, H, W], U8, kind="ExternalOutput").ap()

    with tile.TileContext(nc) as tc, ExitStack() as ctx:
        sb = ctx.enter_context(tc.tile_pool(name="sb", bufs=1))
        xpool = ctx.enter_context(tc.tile_pool(name="xp", bufs=1))
        spool = ctx.enter_context(tc.tile_pool(name="sp", bufs=1))
        ps = ctx.enter_context(tc.tile_pool(name="ps", bufs=1, space="PSUM"))

        # constants: scan reset masks + PE identity
        # fwd: reset at k % 41 == 0 ; rev (suffix, scanned backwards):
        # reset at k % 41 == 40 plus the truncated tail element 1023.
        mf = sb.tile([128, W], F32, name="mf")
        mr = sb.tile([128, W], F32, name="mr")
        idn = sb.tile([128, 128], F32, name="idn")
        jneg = sb.tile([128, 128], F32, name="jneg")
        # mask setup runs on the (otherwise idle) Pool engine, hidden under
        # the first input DMA instead of consuming DVE time; the scan masks
        # come first since the first W-scan gates on them, while idn/jneg
        # are not needed until the first transpose
        nc.gpsimd.memset(mf[:], BIG)
        nc.gpsimd.memset(mf[:, 0:W:WIN], -BIG)
        nc.gpsimd.memset(mr[:], BIG)
        nc.gpsimd.memset(mr[:, V * 2:W:WIN], -BIG)
        nc.gpsimd.memset(mr[:, W - 1:W], -BIG)

        # Persistent scan-output tiles with constant-c margins so each
        # combine is a single full-width STT (no edge-clip ops):
        #   Se_ext = [ c*20 | suffix-scan(1024) ]  -> Se_ext[w] = S[w-20] or c
        #   Pe_ext = [ prefix-scan(1024) | c*20 ]  -> Pe_ext[w+20] = P[w+20] or c
        # Single-buffered: every reader/writer is DVE, so program order
        # already serializes reuse.
        EXT = W + V
        scan_bufs = {}
        for nm in ("Pw", "Sw", "Ph", "Sh"):
            pair = []
            for i in range(2):
                t = sb.tile([128, EXT], F32, name=f"{nm}{i}")
                if nm[0] == "P":
                    nc.gpsimd.memset(t[:, W:EXT], C05)
                else:
                    nc.gpsimd.memset(t[:, 0:V], C05)
                pair.append(t)
            scan_bufs[nm] = pair
        make_identity(nc, idn[:])
        # negated identity: stationary operand for the -x PSUM accumulate
        nc.gpsimd.memset(jneg[:], 0.0)
        nc.gpsimd.affine_select(
            out=jneg[:], in_=jneg[:], compare_op=Alu.not_equal,
            fill=-1.0, base=0, pattern=[[-1, 128]], channel_multiplier=1)

        # segmented scans + combine: out[w] = max(S[w-20], P[w+20], c),
        # window clipping supplied by the constant margins.  Double-buffered
        # by unit parity: unit u+1's scans would otherwise WAR-stall on unit
        # u's combine reading the same tiles (~140ns sem latency per op).
        ucnt = [0]

        def vh_pass(out_ap, data_ap, axis):
            e = nc.vector
            ucnt[0] += 1
            u = ucnt[0]
            Pe = scan_bufs["Pw" if axis == "w" else "Ph"][u % 2]
            Se = scan_bufs["Sw" if axis == "w" else "Sh"][u % 2]
            e.tensor_tensor_scan(Pe[:, 0:W], mf[:], data_ap,
                                 -BIG, op0=Alu.min, op1=Alu.max)
            e.tensor_tensor_scan(Se[:, V:EXT][:, ::-1], mr[:, ::-1],
                                 data_ap[:, ::-1],
                                 -BIG, op0=Alu.min, op1=Alu.max)
            e.scalar_tensor_tensor(out_ap, Se[:, 0:W], C05, Pe[:, V:EXT],
                                   op0=Alu.max, op1=Alu.max)

        # ---- per-phase emission state ----
        # strips keyed (img, ch) -> {wb: tile}; slot index a = img*C + ch
        STRIP_BUFS = 3
        strips_u = {}
        strip_last = {}        # (a, wb) -> last reader instruction
        xc_tiles = {}          # (img, hb) -> tile

        def w_phase(img):
            """DVE W-scans/combines + PE transposes + ACT copies to strips."""
            for ch in range(C):
                strips_u[(img, ch)] = {
                    wb: spool.tile([128, W], F32, name=f"st{img}_{ch}_{wb}",
                                   tag=f"st{wb}", bufs=STRIP_BUFS)
                    for wb in range(WB)}
            for hb in range(HB):
                for ch in range(C):
                    xt = xpool.tile([128, W], F32, name=f"xt{img}_{hb}_{ch}",
                                    tag="xt", bufs=4)
                    R = sb.tile([128, W], F32, name="R", tag="R", bufs=3)
                    if img == 0 and hb == 0 and ch == 0:
                        # first unit: split the load in half and chain two
                        # half-scans per direction so DVE starts after only
                        # half the first DMA (which gates the whole stream).
                        # Finer splits lose: HWDGE desc-gen serializes per
                        # dma_start (~625ns), delaying the last chunk.
                        # asymmetric split s=560: the first-chunk scan ends
                        # just before the second chunk's sem gate, minimizing
                        # the serial chain after it
                        S1 = 560
                        nc.sync.dma_start(xt[:, 0:S1],
                                          xp_d[img, ch, 0:128, 0:S1])
                        nc.sync.dma_start(xt[:, S1:W],
                                          xp_d[img, ch, 0:128, S1:W])
                        e = nc.vector
                        ucnt[0] += 1
                        u = ucnt[0]
                        Pe = scan_bufs["Pw"][u % 2]
                        Se = scan_bufs["Sw"][u % 2]
                        e.tensor_tensor_scan(
                            Pe[:, 0:S1], mf[:, 0:S1], xt[:, 0:S1],
                            -BIG, op0=Alu.min, op1=Alu.max)
                        e.tensor_tensor_scan(
                            Pe[:, S1:W], mf[:, S1:W], xt[:, S1:W],
                            Pe[:, S1 - 1:S1], op0=Alu.min, op1=Alu.max)
                        e.tensor_tensor_scan(
                            Se[:, V + S1:EXT][:, ::-1], mr[:, S1:W][:, ::-1],
                            xt[:, S1:W][:, ::-1],
                            -BIG, op0=Alu.min, op1=Alu.max)
                        e.tensor_tensor_scan(
                            Se[:, V:V + S1][:, ::-1], mr[:, 0:S1][:, ::-1],
                            xt[:, 0:S1][:, ::-1],
                            Se[:, V + S1:V + S1 + 1], op0=Alu.min, op1=Alu.max)
                        e.scalar_tensor_tensor(
                            R[:], Se[:, 0:W], C05, Pe[:, V:EXT],
                            op0=Alu.max, op1=Alu.max)
                    else:
                        nc.sync.dma_start(
                            xt[:], xp_d[img, ch, hb * 128:(hb + 1) * 128])
                        vh_pass(R[:], xt[:], "w")
                    for wb in range(WB):
                        pt = ps.tile([128, 128], F32, name="pt", tag="pt", bufs=3)
                        nc.tensor.transpose(pt[:], R[:, wb * 128:(wb + 1) * 128],
                                            idn[:])
                        cp = nc.scalar.copy(
                            strips_u[(img, ch)][wb][:, hb * 128:(hb + 1) * 128],
                            pt[:])
                        if hb == 0:
                            key = (img * C + ch - STRIP_BUFS, wb)
                            if key in strip_last:
                                _add_dep_helper(cp.ins, strip_last[key].ins,
                                                True, "strip slot reuse")

        def h_phase(img, ch):
            """DVE H-scans/combines, M2_T written in-place over the strips."""
            for wb in range(WB):
                st = strips_u[(img, ch)][wb]
                vh_pass(st[:, 0:W], st[:], "h")

        def f_phase(img, ch, dve_cmp=False):
            """Transpose M2_T back; PE accumulates -x so PSUM = M2 - x and
            ACT emits the mask as relu((M2-x) * -3e38 + 1) -> u8.  The very
            last unit keeps the compare on DVE instead: its transposes are
            gated on the final H-combines, so a PE+ACT tail there would run
            after DVE drains."""
            for hb in range(HB):
                # per-phase planar x reload: contiguous per-channel rows;
                # sharing one load between both channel phases makes a later
                # phase read a pool slot that an earlier phase's prefetch
                # needs freed -> PE-order cycle (deadlock).
                xc = sb.tile([128, W], F32, name=f"xc{img}_{ch}_{hb}",
                             tag="xc", bufs=6)
                nc.sync.dma_start(xc[:],
                                  xp_d[img, ch, hb * 128:(hb + 1) * 128])
                ot = sb.tile([128, W], U8, name=f"ot{img}_{ch}_{hb}",
                             tag=f"ot{hb}", bufs=2)
                m2p = ps.tile([128, W], F32, name="m2p", tag="m2p", bufs=2)
                for wb in range(WB):
                    tb = nc.tensor.matmul(
                        m2p[:, wb * 128:(wb + 1) * 128],
                        strips_u[(img, ch)][wb][:, hb * 128:(hb + 1) * 128],
                        idn[:], is_transpose=True,
                        start=True, stop=dve_cmp)
                    if not dve_cmp:
                        # PSUM block += -x  =>  holds M2 - x (sign-exact)
                        nc.tensor.matmul(
                            m2p[:, wb * 128:(wb + 1) * 128],
                            jneg[:],
                            xc[:, wb * 128:(wb + 1) * 128],
                            start=False, stop=True)
                    if hb == HB - 1:
                        strip_last[(img * C + ch, wb)] = tb
                if dve_cmp:
                    nc.vector.tensor_tensor(
                        ot[:], xc[:], m2p[:], op=Alu.is_ge)
                else:
                    nc.scalar.activation(
                        ot[:], m2p[:],
                        mybir.ActivationFunctionType.Relu,
                        bias=1.0, scale=-3e38)
                # stores go out on the ACT DGE queue: sharing the SP queue
                # with the loads head-of-line-blocks later input DMAs behind
                # stores gated on the compute chain.
                nc.scalar.dma_start(
                    y_d[img, ch, hb * 128:(hb + 1) * 128], ot[:])

        # Software-pipelined emission: each engine's program order matches
        # dependency readiness, so no engine head-of-line-blocks another.
        #   DVE: W0 | H0c0 | H0c1 | W1 | H1c0 | H1c1 | cmp
        #   PE : T0 | F0c0 | T1 | F0c1 | F1c0 | F1c1
        w_phase(0)
        h_phase(0, 0)
        f_phase(0, 0)
        h_phase(0, 1)
        w_phase(1)
        f_phase(0, 1)
        h_phase(1, 0)
        f_phase(1, 0)
        h_phase(1, 1)
        f_phase(1, 1, dve_cmp=True)

    nc.compile()
    return nc


def _get_nc():
    if "nc" not in _CACHE:
        _CACHE["nc"] = _build()
    return _CACHE["nc"]


def _install_neff_cache():
    """Cache compiled NEFFs on disk keyed by BIR hash (compile is ~10 min)."""
    if _CACHE.get("neff_cache"):
        return
    import hashlib
    import shutil
    from concourse import bass_utils, bass2jax

    real = bass_utils.compile_bir_kernel
    cache_dir = "/tmp/bass_neff_cache"

    def cached(bir_json, tmpdir, neff_name="file.neff"):
        os.makedirs(cache_dir, exist_ok=True)
        key = hashlib.sha256(bir_json).hexdigest()[:32]
        hit = os.path.join(cache_dir, key + ".neff")
        dst = os.path.join(tmpdir, neff_name)
        if os.path.exists(hit):
            shutil.copyfile(hit, dst)
            return dst
        out = real(bir_json, tmpdir, neff_name)
        try:
            shutil.copyfile(out, hit)
        except OSError:
            pass
        return out

    bass_utils.compile_bir_kernel = cached
    if getattr(bass2jax, "compile_bir_kernel", None) is not None:
        bass2jax.compile_bir_kernel = cached
    _CACHE["neff_cache"] = True


def kernel(batch_heatmap: np.ndarray) -> np.ndarray:
    if _TRN_REPO not in sys.path:
        sys.path.insert(0, _TRN_REPO)
    from concourse.bass_utils import run_bass_kernel_spmd
    _install_neff_cache()

    x = np.asarray(batch_heatmap, dtype=np.float32)
    assert x.shape == (16, H, W, C), x.shape
    xp = np.ascontiguousarray(x.transpose(0, 3, 1, 2))  # (16, C, H, W)
    nc = _get_nc()
    in_maps = [
        {"xp": xp[B_PER_CORE * r:B_PER_CORE * (r + 1)]}
        for r in range(N_CORES)
    ]
    res = run_bass_kernel_spmd(nc, in_maps, list(range(N_CORES)))
    out = np.stack([res.results[r]["y"] for r in range(N_CORES)])
    # device output is channel-planar (16, C, H, W) -> (16, H, W, C)
    return out.reshape(16, C, H, W).transpose(0, 2, 3, 1).astype(bool)

